# revision 4
# baseline (speedup 1.0000x reference)
"""Trainium2 Bass kernel for the low-rank GRU problem.

Strategy: data-parallel over batch across 8 NeuronCores (32 rows/core).
Per core, state x is kept transposed in SBUF as [128 partitions, 4*32]
(h = q*128 + p, free index = q*32 + b).  Input projections are computed
on-chip in 8-step chunks (one big matmul per (gate, h-tile) into PSUM);
the per-step low-rank recurrence matmuls accumulate on top of the
projections in PSUM, and the activations read PSUM directly.  All static
scales (1/H, NOISE_STD, TAU) are folded into host-side weight/noise prep.
"""

import sys

for _p in ("/opt/trn_rl_repo",):
    if _p not in sys.path:
        sys.path.insert(0, _p)

import numpy as np

B, S, H, R, I, O = 256, 512, 512, 8, 128, 64
TAU, NOISE_STD = 0.2, 0.05
N_CORES = 8
BL = B // N_CORES          # 32 batch rows per core
NQ = H // 128              # 4 h-tiles
SC = 8                     # steps per chunk

_CACHE = {}


def _build(s_total):
    """Build the single-core SPMD Bass program (same program on all 8 cores)."""
    import concourse.bass as bass
    import concourse.tile as tile
    from concourse import bacc, mybir

    f32 = mybir.dt.float32
    Sig = mybir.ActivationFunctionType.Sigmoid
    Tnh = mybir.ActivationFunctionType.Tanh
    MUL = mybir.AluOpType.mult
    ADD = mybir.AluOpType.add

    nchunk = s_total // SC
    nc = bacc.Bacc("TRN2", target_bir_lowering=False, debug=False,
                   num_devices=N_CORES)

    u_pre = nc.dram_tensor("u_pre", [128, s_total * BL], f32, kind="ExternalInput")
    noise_pre = nc.dram_tensor("noise_pre", [128, s_total * 128], f32, kind="ExternalInput")
    x0_pre = nc.dram_tensor("x0_pre", [128, NQ * BL], f32, kind="ExternalInput")
    wproj = nc.dram_tensor("wproj", [128, 12 * 128], f32, kind="ExternalInput")
    n_allw = nc.dram_tensor("n_allw", [128, NQ * 24], f32, kind="ExternalInput")
    mbw = nc.dram_tensor("mbw", [8, 12 * 128], f32, kind="ExternalInput")
    woutw = nc.dram_tensor("woutw", [128, NQ * O], f32, kind="ExternalInput")

    traj_out = nc.dram_tensor("traj_out", [128, s_total * 128], f32, kind="ExternalOutput")
    out_pre = nc.dram_tensor("out_pre", [O, s_total * BL], f32, kind="ExternalOutput")
    x_final = nc.dram_tensor("x_final", [128, NQ * BL], f32, kind="ExternalOutput")

    with tile.TileContext(nc) as tc:
        with (
            tc.tile_pool(name="const", bufs=1) as constp,
            tc.tile_pool(name="io", bufs=2) as iop,
            tc.tile_pool(name="trajp", bufs=2) as trajp,
            tc.tile_pool(name="work", bufs=2) as workp,
            tc.tile_pool(name="pproj", bufs=1, space="PSUM") as pprojp,
            tc.tile_pool(name="pv", bufs=1, space="PSUM") as pvp,
            tc.tile_pool(name="pout", bufs=1, space="PSUM") as poutp,
        ):
            # ---- resident weights ----
            wproj_sb = constp.tile([128, 12 * 128], f32)
            nc.sync.dma_start(out=wproj_sb, in_=wproj[:, :])
            n_all_sb = constp.tile([128, NQ * 24], f32)
            nc.sync.dma_start(out=n_all_sb, in_=n_allw[:, :])
            mb_sb = constp.tile([8, 12 * 128], f32)
            nc.sync.dma_start(out=mb_sb, in_=mbw[:, :])
            wout_sb = constp.tile([128, NQ * O], f32)
            nc.sync.dma_start(out=wout_sb, in_=woutw[:, :])
            x0_sb = constp.tile([128, NQ * BL], f32)
            nc.sync.dma_start(out=x0_sb, in_=x0_pre[:, :])

            traj_prev = None
            for c in range(nchunk):
                u_ch = iop.tile([128, SC * BL], f32, tag="u_ch")
                nc.sync.dma_start(out=u_ch, in_=u_pre[:, c * SC * BL:(c + 1) * SC * BL])
                n_ch = iop.tile([128, SC * 128], f32, tag="n_ch")
                nc.sync.dma_start(out=n_ch, in_=noise_pre[:, c * SC * 128:(c + 1) * SC * 128])

                # PSUM proj tiles: [128, q, s, b]
                p_r = pprojp.tile([128, NQ, SC, BL], f32, tag="p_r")
                p_z = pprojp.tile([128, NQ, SC, BL], f32, tag="p_z")
                p_h = pprojp.tile([128, NQ, SC, BL], f32, tag="p_h")
                # start=True clears has_written for the WHOLE bank; q=0,1 share
                # bank 0 and q=2,3 bank 1, so only the first matmul touching
                # each bank may use start=True (else stage-B accumulation
                # overwrites instead of adding).
                for g, p_g in enumerate((p_r, p_z, p_h)):
                    for q in range(NQ):
                        nc.tensor.matmul(
                            out=p_g[:, q],
                            lhsT=wproj_sb[:, (g * NQ + q) * 128:(g * NQ + q + 1) * 128],
                            rhs=u_ch,
                            start=(q % 2 == 0), stop=(q % 2 == 1),
                            skip_group_check=True,
                        )

                traj_ch = trajp.tile([128, SC, 128], f32, tag="traj")

                for s in range(SC):
                    if c == 0 and s == 0:
                        x_ap = x0_sb
                    elif s == 0:
                        x_ap = traj_prev[:, SC - 1, :]
                    else:
                        x_ap = traj_ch[:, s - 1, :]

                    # ---- stage A (r, z): v = x @ N  ----
                    pv_r = pvp.tile([8, BL], f32, tag="pv")
                    for q in range(NQ):
                        nc.tensor.matmul(
                            out=pv_r,
                            lhsT=n_all_sb[:, q * 24:q * 24 + 8],
                            rhs=x_ap[:, q * BL:(q + 1) * BL],
                            start=(q == 0), stop=(q == NQ - 1),
                        )
                    vr_sb = workp.tile([8, BL], f32, tag="vr")
                    nc.scalar.copy(vr_sb, pv_r)

                    pv_z = pvp.tile([8, BL], f32, tag="pv")
                    for q in range(NQ):
                        nc.tensor.matmul(
                            out=pv_z,
                            lhsT=n_all_sb[:, q * 24 + 8:q * 24 + 16],
                            rhs=x_ap[:, q * BL:(q + 1) * BL],
                            start=(q == 0), stop=(q == NQ - 1),
                        )
                    vz_sb = workp.tile([8, BL], f32, tag="vz")
                    nc.scalar.copy(vz_sb, pv_z)

                    # ---- stage B (r, z): accumulate M @ v onto projections ----
                    for q in range(NQ):
                        nc.tensor.matmul(
                            out=p_r[:, q, s], lhsT=mb_sb[:, (0 * NQ + q) * 128:(0 * NQ + q + 1) * 128],
                            rhs=vr_sb, start=False, stop=True, skip_group_check=True,
                        )
                    for q in range(NQ):
                        nc.tensor.matmul(
                            out=p_z[:, q, s], lhsT=mb_sb[:, (1 * NQ + q) * 128:(1 * NQ + q + 1) * 128],
                            rhs=vz_sb, start=False, stop=True, skip_group_check=True,
                        )

                    r_sb = workp.tile([128, NQ * BL], f32, tag="r")
                    nc.scalar.activation(r_sb, p_r[:, :, s, :], Sig)
                    z_sb = workp.tile([128, NQ * BL], f32, tag="z")
                    nc.scalar.activation(z_sb, p_z[:, :, s, :], Sig)

                    rx_sb = workp.tile([128, NQ * BL], f32, tag="rx")
                    nc.vector.tensor_mul(rx_sb, r_sb, x_ap)

                    # ---- stage A/B (h-gate) ----
                    pv_h = pvp.tile([8, BL], f32, tag="pv")
                    for q in range(NQ):
                        nc.tensor.matmul(
                            out=pv_h,
                            lhsT=n_all_sb[:, q * 24 + 16:q * 24 + 24],
                            rhs=rx_sb[:, q * BL:(q + 1) * BL],
                            start=(q == 0), stop=(q == NQ - 1),
                        )
                    vh_sb = workp.tile([8, BL], f32, tag="vh")
                    nc.scalar.copy(vh_sb, pv_h)
                    for q in range(NQ):
                        nc.tensor.matmul(
                            out=p_h[:, q, s], lhsT=mb_sb[:, (2 * NQ + q) * 128:(2 * NQ + q + 1) * 128],
                            rhs=vh_sb, start=False, stop=True, skip_group_check=True,
                        )

                    g_sb = workp.tile([128, NQ * BL], f32, tag="g")
                    nc.scalar.activation(g_sb, p_h[:, :, s, :], Tnh)

                    # ---- state update ----
                    # x' = 0.8x + noise' + 0.2 g + 0.2 z (x - g)
                    c1_sb = workp.tile([128, NQ * BL], f32, tag="c1")
                    nc.gpsimd.tensor_sub(c1_sb, x_ap, g_sb)
                    c2_sb = workp.tile([128, NQ * BL], f32, tag="c2")
                    nc.vector.scalar_tensor_tensor(c2_sb, c1_sb, TAU, z_sb, MUL, MUL)
                    c3_sb = workp.tile([128, NQ * BL], f32, tag="c3")
                    nc.vector.scalar_tensor_tensor(
                        c3_sb, x_ap, 1.0 - TAU, n_ch[:, s * 128:(s + 1) * 128], MUL, ADD)
                    c4_sb = workp.tile([128, NQ * BL], f32, tag="c4")
                    nc.vector.scalar_tensor_tensor(c4_sb, g_sb, TAU, c2_sb, MUL, ADD)
                    nc.vector.tensor_add(traj_ch[:, s, :], c3_sb, c4_sb)

                # ---- write traj chunk, output projection ----
                nc.sync.dma_start(
                    out=traj_out[:, c * SC * 128:(c + 1) * SC * 128],
                    in_=traj_ch[:, :, :],
                )
                po = poutp.tile([O, SC * BL], f32, tag="po")
                for q in range(NQ):
                    nc.tensor.matmul(
                        out=po,
                        lhsT=wout_sb[:, q * O:(q + 1) * O],
                        rhs=traj_ch[:, :, q * BL:(q + 1) * BL],
                        start=(q == 0), stop=(q == NQ - 1),
                    )
                o_sb = iop.tile([O, SC * BL], f32, tag="o_sb")
                nc.scalar.copy(o_sb, po)
                nc.sync.dma_start(
                    out=out_pre[:, c * SC * BL:(c + 1) * SC * BL], in_=o_sb)

                traj_prev = traj_ch

            nc.sync.dma_start(out=x_final[:, :], in_=traj_prev[:, SC - 1, :])

    nc.compile()
    return nc


def _prep_shared(inputs):
    """Host-side prep of shared (replicated) weight arrays."""
    Nc = np.concatenate(
        [inputs["N_hr"], inputs["N_hz"], inputs["N_hh"]], axis=1).astype(np.float32) / H
    # n_all[p, q*24 + j] = Nc[q*128+p, j]
    n_all = np.ascontiguousarray(
        Nc.reshape(NQ, 128, 24).transpose(1, 0, 2)).reshape(128, NQ * 24)
    # mb[j, (g*4+q)*128 + m] = M_g[q*128+m, j]
    mb = np.stack(
        [inputs[k].astype(np.float32).reshape(NQ, 128, R).transpose(2, 0, 1)
         for k in ("M_hr", "M_hz", "M_hh")], axis=1).reshape(R, 12 * 128)
    # wproj[i, (g*4+q)*128+m] = W_g[q*128+m, i]
    wp = np.stack(
        [inputs[k].astype(np.float32).reshape(NQ, 128, I).transpose(2, 0, 1)
         for k in ("Wir_w", "Wiz_w", "Wih_w")], axis=1).reshape(I, 12 * 128)
    # wout[p, q*64+o] = Wout[o, q*128+p]
    wout = np.ascontiguousarray(
        inputs["Wout_w"].astype(np.float32).T.reshape(NQ, 128, O).transpose(1, 0, 2)
    ).reshape(128, NQ * O)
    return (np.ascontiguousarray(n_all), np.ascontiguousarray(mb),
            np.ascontiguousarray(wp), np.ascontiguousarray(wout))


def _prep_core(inputs, c, s_total):
    bs = slice(c * BL, (c + 1) * BL)
    u_c = np.ascontiguousarray(
        inputs["u"][bs, :s_total].astype(np.float32).transpose(2, 1, 0)
    ).reshape(I, s_total * BL)
    noise_c = np.ascontiguousarray(
        (inputs["noise"][:s_total, bs, :].astype(np.float32) * NOISE_STD)
        .reshape(s_total, BL, NQ, 128).transpose(3, 0, 2, 1)
    ).reshape(128, s_total * 128)
    x0_c = np.ascontiguousarray(
        inputs["x0"][bs].astype(np.float32).reshape(BL, NQ, 128).transpose(2, 1, 0)
    ).reshape(128, NQ * BL)
    return u_c, noise_c, x0_c


def run(inputs, s_total=S, trace=False, trace_kwargs=None):
    """Build (cached), run on 8 cores, return (results_list, bass_results)."""
    from concourse.bass_utils import run_bass_kernel_spmd

    if s_total not in _CACHE:
        _CACHE[s_total] = _build(s_total)
    nc = _CACHE[s_total]

    n_all, mb, wp, wout = _prep_shared(inputs)
    in_maps = []
    for c in range(N_CORES):
        u_c, noise_c, x0_c = _prep_core(inputs, c, s_total)
        in_maps.append({
            "u_pre": u_c, "noise_pre": noise_c, "x0_pre": x0_c,
            "wproj": wp, "n_allw": n_all, "mbw": mb, "woutw": wout,
        })
    kw = dict(trace_kwargs or {})
    res = run_bass_kernel_spmd(nc, in_maps, list(range(N_CORES)), trace=trace, **kw)
    return res.results, res


def assemble(results, inputs, s_total=S):
    outs, xfs, trajs = [], [], []
    wout_b = inputs["Wout_b"].astype(np.float32)
    for c in range(N_CORES):
        r = results[c]
        traj_c = r["traj_out"].reshape(128, s_total, NQ, BL).transpose(3, 1, 2, 0) \
            .reshape(BL, s_total, H)
        out_c = r["out_pre"].reshape(O, s_total, BL).transpose(2, 1, 0) + wout_b
        xf_c = r["x_final"].reshape(128, NQ, BL).transpose(2, 1, 0).reshape(BL, H)
        trajs.append(traj_c)
        outs.append(out_c)
        xfs.append(xf_c)
    output = np.ascontiguousarray(np.concatenate(outs, 0), dtype=np.float32)
    x_fin = np.ascontiguousarray(np.concatenate(xfs, 0), dtype=np.float32)
    traj = np.ascontiguousarray(np.concatenate(trajs, 0), dtype=np.float32)
    return output, x_fin, traj


def kernel(**inputs):
    inputs = {k: np.asarray(v) for k, v in inputs.items()}
    results, _ = run(inputs)
    return assemble(results, inputs)
